# revision 3
# baseline (speedup 1.0000x reference)
"""Trainium2 Bass kernel for nn_Conv2D_80796924772741.

Depthwise (grouped, F=64) 3x3 valid conv over [F, 514, 514, 4] int8 with
per-channel int8 weights + int32 bias, followed by exact fixed-point requant
  res = (acc * 19920 + 2^21) >> 22 ;  out = clip(res - 5, -128, 127) int8
(reduced_mantissa 19920 = 1245 * 16 -> res = (acc*1245 + 2^17) >> 18).

Sharding: F=64 split across 8 NeuronCores (8 channels each), embarrassingly
parallel.

Per-core compute, per channel f and output-row window (124 rows):
 - PE: Toeplitz-band stationary matmuls over H-windows: contraction
   K = m_r + 4 (2 bias rows driven by all-ones rhs partitions + m_r + 2 input
   rows), all 3 H-taps as band diagonals, 3 matmuls for the 3 W-taps
   (free-dim offset +4n), accumulating a [m_r, 2048] PSUM mega-tile spanning
   4 banks (4 x 512-column chunks).  PSUM acc A = conv + b exactly (all
   intermediates integers < 2^24 in fp32).
 - Requant in ONE op per window, reading the 4-bank PSUM AP directly:
     out = sat_i8(rne(A * C + B0)),  C = f32(1245/2^18),
     B0 = f32(-5 + 4*2^-21)
   ACT (even windows) / DVE (odd windows) for engine balance.  Verified
   bit-exact offline over every A in [-147304, 147304] and on-hardware over
   the same range for both engines (single-rounding fma on ACT,
   double-rounding on DVE; both convert with RNE + saturation).
"""

import numpy as np
import ml_dtypes

F_PER_CORE = 8
H_IN = 514
W_IN = 514
D = 4
H_OUT = 512
WD_OUT = 2048  # 512 * 4
FREE_IN = W_IN * D  # 2056
N_CHUNK = 512
N_CORES = 8
M_WIN = 124  # output rows per window (K = M + 4 <= 128)

WINDOWS = [(0, 124), (124, 124), (248, 124), (372, 124), (496, 16)]

CPF = np.float32(1245.0 / 2.0**18)
B0 = float(np.float32(-5.0 + 4 * 2.0**-21))


def _build_lhsT(w_core: np.ndarray, b_core: np.ndarray) -> np.ndarray:
    """[128, 8*3*124] bf16 stationary: per (channel, w-tap) a Toeplitz band.

    Column block (f*3 + n)*124 : +124 holds T_n for channel f:
      T_n[2 + i + m, i] = w[f, m, n]   (rows 2.. are conv data partitions)
      T_0[0, i] = 8*floor(b/8) ; T_0[1, i] = b mod 8   (bias rows, driven by
      all-ones rhs partitions 0/1; both values exact in bf16).
    """
    out = np.zeros((128, F_PER_CORE * 3 * M_WIN), dtype=np.float32)
    idx = np.arange(M_WIN)
    for f in range(F_PER_CORE):
        b_f = int(b_core[f])
        bh = b_f >> 3
        bl = b_f - 8 * bh
        for n in range(3):
            base = (f * 3 + n) * M_WIN
            if n == 0:
                out[0, base : base + M_WIN] = float(8 * bh)
                out[1, base : base + M_WIN] = float(bl)
            for m in range(3):
                out[2 + idx + m, base + idx] = float(int(w_core[f, m, n, 0]))
    return out.astype(ml_dtypes.bfloat16)


_PROGRAM_CACHE = {}


def _build_program():
    import concourse.bass as bass
    import concourse.tile as tile
    from concourse import bacc, mybir

    nc = bacc.Bacc(
        "TRN2", target_bir_lowering=False, debug=False, num_devices=N_CORES
    )
    dt = mybir.dt
    Alu = mybir.AluOpType
    Act = mybir.ActivationFunctionType

    x_d = nc.dram_tensor(
        "x", [F_PER_CORE, H_IN, FREE_IN], dt.int8, kind="ExternalInput"
    ).ap()
    lhsT_d = nc.dram_tensor(
        "lhsT", [128, F_PER_CORE * 3 * M_WIN], dt.bfloat16, kind="ExternalInput"
    ).ap()
    ones_d = nc.dram_tensor("ones2", [2, FREE_IN], dt.bfloat16, kind="ExternalInput").ap()
    y_d = nc.dram_tensor(
        "y", [F_PER_CORE, H_OUT, WD_OUT], dt.int8, kind="ExternalOutput"
    ).ap()

    with tile.TileContext(nc) as tc:
        with (
            tc.tile_pool(name="const", bufs=1) as const_pool,
            tc.tile_pool(name="xin", bufs=3) as x_pool,
            tc.tile_pool(name="psum", bufs=2, space="PSUM") as psum_pool,
            tc.tile_pool(name="otile", bufs=3) as o_pool,
        ):
            lhsT_t = const_pool.tile([128, F_PER_CORE * 3 * M_WIN], dt.bfloat16)
            nc.sync.dma_start(lhsT_t[:], lhsT_d[:])

            widx = 0
            for f in range(F_PER_CORE):
                for (r0, m_r) in WINDOWS:
                    k_r = m_r + 4
                    xt = x_pool.tile([128, FREE_IN], dt.bfloat16)
                    # bias-driving all-ones rows
                    nc.sync.dma_start(xt[0:2, :], ones_d[:])
                    # data rows with int8 -> bf16 cast (SWDGE)
                    nc.gpsimd.dma_start(
                        xt[2 : 2 + m_r + 2, :], x_d[f, r0 : r0 + m_r + 2, :]
                    )
                    ps = psum_pool.tile([M_WIN, WD_OUT], dt.float32)
                    for n in range(3):
                        base = (f * 3 + n) * M_WIN
                        for c in range(4):
                            nc.tensor.matmul(
                                ps[0:m_r, c * N_CHUNK : (c + 1) * N_CHUNK],
                                lhsT_t[0:k_r, base : base + m_r],
                                xt[0:k_r, 4 * n + c * N_CHUNK : 4 * n + c * N_CHUNK + N_CHUNK],
                                start=(n == 0),
                                stop=(n == 2),
                                skip_group_check=True,
                            )
                    ot = o_pool.tile([M_WIN, WD_OUT], dt.int8)
                    if widx % 2 == 0:
                        nc.scalar.activation(
                            ot[0:m_r, :], ps[0:m_r, :], Act.Copy,
                            bias=B0, scale=float(CPF),
                        )
                    else:
                        nc.vector.tensor_scalar(
                            ot[0:m_r, :], ps[0:m_r, :], float(CPF), B0,
                            Alu.mult, Alu.add,
                        )
                    nc.sync.dma_start(
                        y_d[f, r0 : r0 + m_r, :], ot[0:m_r, :]
                    )
                    widx += 1

    nc.compile()
    return nc


def kernel(x: np.ndarray, w: np.ndarray, b: np.ndarray) -> np.ndarray:
    """x: int8 [64, 514, 514, 4]; w: int8 [64, 3, 3, 1]; b: int32 [64].

    Returns int8 [64, 512, 512, 4].
    """
    from concourse.bass_utils import run_bass_kernel_spmd

    if "nc" not in _PROGRAM_CACHE:
        _PROGRAM_CACHE["nc"] = _build_program()
    nc = _PROGRAM_CACHE["nc"]

    F = x.shape[0]
    assert F == N_CORES * F_PER_CORE

    ones2 = np.ones((2, FREE_IN), dtype=np.float32).astype(ml_dtypes.bfloat16)

    in_maps = []
    for core in range(N_CORES):
        lo = core * F_PER_CORE
        hi = lo + F_PER_CORE
        x_shard = np.ascontiguousarray(x[lo:hi]).reshape(F_PER_CORE, H_IN, FREE_IN)
        lhsT = _build_lhsT(w[lo:hi], b[lo:hi])
        in_maps.append({"x": x_shard, "lhsT": lhsT, "ones2": ones2})

    res = run_bass_kernel_spmd(nc, in_maps, core_ids=list(range(N_CORES)))

    out = np.empty((F, H_OUT, 512, D), dtype=np.int8)
    for core in range(N_CORES):
        lo = core * F_PER_CORE
        y = res.results[core]["y"]  # [8, 512, 2048] int8
        out[lo : lo + F_PER_CORE] = y.reshape(F_PER_CORE, H_OUT, 512, D)
    return out


# revision 5
# speedup vs baseline: 1.0174x; 1.0174x over previous
"""Trainium2 Bass kernel for nn_Conv2D_80796924772741.

Depthwise (grouped, F=64) 3x3 valid conv over [F, 514, 514, 4] int8 with
per-channel int8 weights + int32 bias, followed by exact fixed-point requant
  res = (acc * 19920 + 2^21) >> 22 ;  out = clip(res - 5, -128, 127) int8
(reduced_mantissa 19920 = 1245 * 16 -> res = (acc*1245 + 2^17) >> 18).

Sharding: F=64 split across 8 NeuronCores (8 channels each), embarrassingly
parallel.

Per-core structure (v2):
 - x is loaded window-at-a-time for ALL 8 channels in ONE SWDGE cast-DMA
   (int8 -> bf16): tile [128, 8*2056], partitions 0..1 all-ones (drive the
   bias rows), partitions 2..127 the 126 input rows of the window, channel f
   in free block f*2056.  This amortizes the ~2us fixed SWDGE cost.
 - PE: per (channel, window) a [128-row, 2048] PSUM mega-tile (4 banks) is
   accumulated by 12 matmuls: Toeplitz-band stationary lhsT (3 H-taps as
   band diagonals, K = 128 incl. 2 bias rows), x 3 W-taps (free offset +4n)
   x 4 512-column chunks.  Bands are padded to 128 columns (FWL).
 - The last 16 output rows of all channels are packed 4-channels-per-matmul
   via block-diagonal bands ([74, 64] with shared ones rows), 2 groups.
 - Requant in ONE op per (channel, window), reading the 4-bank PSUM AP:
     out = sat_i8(rne(A * C + B0)),  C = f32(1245/2^18), B0 = f32(-5+4*2^-21)
   alternating ACT / DVE (they read different PSUM buffers -> different
   banks, so they run in parallel).  Verified bit-exact offline over every
   A in [-147304, 147304] and on-hardware over the same range on both
   engines (single-rounding fma on ACT, double-rounding on DVE; both
   convert with RNE + saturation).
"""

import numpy as np
import ml_dtypes

F_PER_CORE = 8
H_IN = 514
W_IN = 514
D = 4
H_OUT = 512
WD_OUT = 2048  # 512 * 4
FREE_IN = W_IN * D  # 2056
N_CHUNK = 512
N_CORES = 8
M_WIN = 124
MAIN_WINDOWS = [(0, 124), (124, 124), (248, 124), (372, 124)]
R0_REM = 496  # remainder: output rows 496..511, 16 per channel
X_BLOCK = FREE_IN  # per-channel free block in the window tile

LHST_MAIN = 128  # padded band width (FWL) per (f, n) block
LHST_REM = 64  # block-diag remainder width per (group, n) block
LHST_COLS = F_PER_CORE * 3 * LHST_MAIN + 2 * 3 * LHST_REM  # 3456

CPF = np.float32(1245.0 / 2.0**18)
B0 = float(np.float32(-5.0 + 4 * 2.0**-21))


def _build_lhsT(w_core: np.ndarray, b_core: np.ndarray) -> np.ndarray:
    """[128, LHST_COLS] bf16 stationary weights.

    Main block (f*3 + n)*128 : +128 (cols 0..123 used):
      T[0, i] = 8*floor(b_f/8) if n == 0 else 0 ; T[1, i] = b_f mod 8 (n==0)
      T[2 + i + m, i] = w[f, m, n]
    Remainder block 3072 + (t*3 + n)*64 : +64  (t = channel group 0/1):
      col 16g + i (g = 0..3 channel in group, i = 0..15 output row):
        T[0, col] = 8*floor(b/8) (n==0), T[1, col] = b mod 8 (n==0)
        T[2 + 18g + i + m, col] = w[4t + g, m, n]
    """
    out = np.zeros((128, LHST_COLS), dtype=np.float32)
    idx = np.arange(M_WIN)
    for f in range(F_PER_CORE):
        b_f = int(b_core[f])
        bh, bl = b_f >> 3, b_f - 8 * (b_f >> 3)
        for n in range(3):
            base = (f * 3 + n) * LHST_MAIN
            if n == 0:
                out[0, base : base + M_WIN] = float(8 * bh)
                out[1, base : base + M_WIN] = float(bl)
            for m in range(3):
                out[2 + idx + m, base + idx] = float(int(w_core[f, m, n, 0]))
    i16 = np.arange(16)
    for t in range(2):
        for n in range(3):
            base = F_PER_CORE * 3 * LHST_MAIN + (t * 3 + n) * LHST_REM
            for g in range(4):
                f = 4 * t + g
                b_f = int(b_core[f])
                bh, bl = b_f >> 3, b_f - 8 * (b_f >> 3)
                col = base + 16 * g
                if n == 0:
                    out[0, col : col + 16] = float(8 * bh)
                    out[1, col : col + 16] = float(bl)
                for m in range(3):
                    out[2 + 18 * g + i16 + m, col + i16] = float(int(w_core[f, m, n, 0]))
    return out.astype(ml_dtypes.bfloat16)


_PROGRAM_CACHE = {}


def _build_program():
    import concourse.bass as bass
    import concourse.tile as tile
    from concourse import bacc, mybir

    nc = bacc.Bacc(
        "TRN2", target_bir_lowering=False, debug=False, num_devices=N_CORES
    )
    dt = mybir.dt
    Alu = mybir.AluOpType
    Act = mybir.ActivationFunctionType

    x_d = nc.dram_tensor(
        "x", [F_PER_CORE, H_IN, FREE_IN], dt.int8, kind="ExternalInput"
    ).ap()
    lhsT_d = nc.dram_tensor(
        "lhsT", [128, LHST_COLS], dt.bfloat16, kind="ExternalInput"
    ).ap()
    ones_d = nc.dram_tensor(
        "ones2", [2, F_PER_CORE * X_BLOCK], dt.bfloat16, kind="ExternalInput"
    ).ap()
    y_d = nc.dram_tensor(
        "y", [F_PER_CORE, H_OUT, WD_OUT], dt.int8, kind="ExternalOutput"
    ).ap()

    with tile.TileContext(nc) as tc:
        with (
            tc.tile_pool(name="const", bufs=1) as const_pool,
            tc.tile_pool(name="xin", bufs=2) as x_pool,
            tc.tile_pool(name="xrem", bufs=2) as xr_pool,
            tc.tile_pool(name="psum", bufs=2, space="PSUM") as psum_pool,
            tc.tile_pool(name="otile", bufs=4) as o_pool,
        ):
            lhsT_t = const_pool.tile([128, LHST_COLS], dt.bfloat16)
            nc.sync.dma_start(lhsT_t[:], lhsT_d[:])

            def requant_store(ps, m_lo, m_hi, widx, dst_ap):
                """One-op requant of ps[m_lo:m_hi, :] -> int8 tile -> DMA out."""
                ot = o_pool.tile([128, WD_OUT], dt.int8)
                if widx % 2 == 0:
                    nc.scalar.activation(
                        ot[m_lo:m_hi, :], ps[m_lo:m_hi, :], Act.Copy,
                        bias=B0, scale=float(CPF),
                    )
                else:
                    nc.vector.tensor_scalar(
                        ot[m_lo:m_hi, :], ps[m_lo:m_hi, :], float(CPF), B0,
                        Alu.mult, Alu.add,
                    )
                nc.sync.dma_start(dst_ap, ot[m_lo:m_hi, :])

            widx = 0
            for (r0, m_r) in MAIN_WINDOWS:
                xt = x_pool.tile([128, F_PER_CORE * X_BLOCK], dt.bfloat16)
                nc.sync.dma_start(xt[0:2, :], ones_d[:])
                # one SWDGE cast-DMA for all 8 channels of this window:
                # dst partition 2+p, free f*2056 + j  <-  x[f, r0+p, j]
                nc.gpsimd.dma_start(
                    xt[2 : 2 + m_r + 2, :],
                    x_d[:, r0 : r0 + m_r + 2, :].transpose([1, 0, 2]),
                )
                for f in range(F_PER_CORE):
                    ps = psum_pool.tile([128, WD_OUT], dt.float32)
                    for n in range(3):
                        base = (f * 3 + n) * LHST_MAIN
                        xoff = f * X_BLOCK + 4 * n
                        for c in range(4):
                            nc.tensor.matmul(
                                ps[:, c * N_CHUNK : (c + 1) * N_CHUNK],
                                lhsT_t[:, base : base + LHST_MAIN],
                                xt[:, xoff + c * N_CHUNK : xoff + c * N_CHUNK + N_CHUNK],
                                start=(n == 0),
                                stop=(n == 2),
                                skip_group_check=True,
                            )
                    requant_store(
                        ps, 0, m_r, widx, y_d[f, r0 : r0 + m_r, :]
                    )
                    widx += 1

            # remainder: output rows 496..511, 4 channels per matmul group
            for t in range(2):
                xr = xr_pool.tile([74, FREE_IN], dt.bfloat16)
                nc.sync.dma_start(xr[0:2, :], ones_d[0:2, 0:FREE_IN])
                # dst partition 2 + 18g + p  <-  x[4t+g, 496+p, :]
                nc.gpsimd.dma_start(
                    xr[2:74, :],
                    x_d[4 * t : 4 * t + 4, R0_REM : R0_REM + 18, :],
                )
                ps = psum_pool.tile([128, WD_OUT], dt.float32)
                for n in range(3):
                    base = F_PER_CORE * 3 * LHST_MAIN + (t * 3 + n) * LHST_REM
                    for c in range(4):
                        nc.tensor.matmul(
                            ps[0:LHST_REM, c * N_CHUNK : (c + 1) * N_CHUNK],
                            lhsT_t[0:74, base : base + LHST_REM],
                            xr[:, 4 * n + c * N_CHUNK : 4 * n + c * N_CHUNK + N_CHUNK],
                            start=(n == 0),
                            stop=(n == 2),
                            skip_group_check=True,
                        )
                # one out-DMA: dst y[4t+g, 496+i, :] <- ot[16g+i, :]
                requant_store(
                    ps, 0, LHST_REM, widx,
                    y_d[4 * t : 4 * t + 4, R0_REM + 16 - 16 : H_OUT, :],
                )
                widx += 1

    nc.compile()
    return nc


def make_in_maps(x: np.ndarray, w: np.ndarray, b: np.ndarray):
    ones2 = np.ones((2, F_PER_CORE * X_BLOCK), dtype=np.float32).astype(
        ml_dtypes.bfloat16
    )
    in_maps = []
    for core in range(N_CORES):
        lo = core * F_PER_CORE
        hi = lo + F_PER_CORE
        x_shard = np.ascontiguousarray(x[lo:hi]).reshape(F_PER_CORE, H_IN, FREE_IN)
        lhsT = _build_lhsT(w[lo:hi], b[lo:hi])
        in_maps.append({"x": x_shard, "lhsT": lhsT, "ones2": ones2})
    return in_maps


def kernel(x: np.ndarray, w: np.ndarray, b: np.ndarray) -> np.ndarray:
    """x: int8 [64, 514, 514, 4]; w: int8 [64, 3, 3, 1]; b: int32 [64].

    Returns int8 [64, 512, 512, 4].
    """
    from concourse.bass_utils import run_bass_kernel_spmd

    if "nc" not in _PROGRAM_CACHE:
        _PROGRAM_CACHE["nc"] = _build_program()
    nc = _PROGRAM_CACHE["nc"]

    F = x.shape[0]
    assert F == N_CORES * F_PER_CORE

    res = run_bass_kernel_spmd(
        nc, make_in_maps(x, w, b), core_ids=list(range(N_CORES))
    )

    out = np.empty((F, H_OUT, 512, D), dtype=np.int8)
    for core in range(N_CORES):
        lo = core * F_PER_CORE
        y = res.results[core]["y"]  # [8, 512, 2048] int8
        out[lo : lo + F_PER_CORE] = y.reshape(F_PER_CORE, H_OUT, 512, D)
    return out


# revision 6
# speedup vs baseline: 1.0564x; 1.0384x over previous
"""Trainium2 Bass kernel for nn_Conv2D_80796924772741.

Depthwise (grouped, F=64) 3x3 valid conv over [F, 514, 514, 4] int8 with
per-channel int8 weights + int32 bias, followed by exact fixed-point requant
  res = (acc * 19920 + 2^21) >> 22 ;  out = clip(res - 5, -128, 127) int8
(reduced_mantissa 19920 = 1245 * 16 -> res = (acc*1245 + 2^17) >> 18).

Sharding: F=64 split across 8 NeuronCores (8 channels each), embarrassingly
parallel.

Per-core structure (v3):
 - x is loaded window-at-a-time for ALL 8 channels in ONE plain int8 SWDGE
   DMA (the int8->bf16 cast-during-DMA path measured only ~14 GB/s/engine,
   so the cast is done on-chip instead): int8 tile [128, 8*2056], partitions
   0..1 all-ones (int8 ones input; they drive the bias rows), partitions
   2..127 the 126 window rows, channel f at free block f*2056.
 - One whole-tile cast op int8 -> bf16 per window, alternating ACT / DVE.
 - PE: per (channel, window) a [128-row, 2048] PSUM mega-tile (4 banks)
   accumulated by 12 matmuls: Toeplitz-band stationary lhsT (3 H-taps as
   band diagonals, K = 128 incl. 2 bias rows), 3 W-taps as rhs free offset
   +4n, 4 x 512-column chunks.  Bands padded to 128 columns (FWL).
 - Last 16 output rows: 4 channels packed per matmul via block-diagonal
   bands ([74, 64] with shared ones rows), 2 groups.
 - Requant in ONE op per (channel, window) reading the 4-bank PSUM AP:
     out = sat_i8(rne(A * C + B0)),  C = f32(1245/2^18), B0 = f32(-5+4*2^-21)
   alternating ACT / DVE.  Verified bit-exact offline over every
   A in [-147304, 147304] and on-hardware on both engines.
 - Output DMAs alternate SWDGE (gpsimd) / HWDGE (sync) so the y-writes
   spread across all 16 SDMA engines (HWDGE alone was observed to use 4).
"""

import numpy as np
import ml_dtypes

F_PER_CORE = 8
H_IN = 514
W_IN = 514
D = 4
H_OUT = 512
WD_OUT = 2048  # 512 * 4
FREE_IN = W_IN * D  # 2056
N_CHUNK = 512
N_CORES = 8
M_WIN = 124
MAIN_WINDOWS = [(0, 124), (124, 124), (248, 124), (372, 124)]
R0_REM = 496  # remainder: output rows 496..511, 16 per channel
X_BLOCK = FREE_IN
XT_COLS = F_PER_CORE * X_BLOCK  # 16448

LHST_MAIN = 128  # padded band width (FWL) per (f, n) block
LHST_REM = 64  # block-diag remainder width per (group, n) block
LHST_COLS = F_PER_CORE * 3 * LHST_MAIN + 2 * 3 * LHST_REM  # 3456

CPF = np.float32(1245.0 / 2.0**18)
B0 = float(np.float32(-5.0 + 4 * 2.0**-21))


def _build_lhsT(w_core: np.ndarray, b_core: np.ndarray) -> np.ndarray:
    """[128, LHST_COLS] bf16 stationary weights (see module docstring)."""
    out = np.zeros((128, LHST_COLS), dtype=np.float32)
    idx = np.arange(M_WIN)
    for f in range(F_PER_CORE):
        b_f = int(b_core[f])
        bh, bl = b_f >> 3, b_f - 8 * (b_f >> 3)
        for n in range(3):
            base = (f * 3 + n) * LHST_MAIN
            if n == 0:
                out[0, base : base + M_WIN] = float(8 * bh)
                out[1, base : base + M_WIN] = float(bl)
            for m in range(3):
                out[2 + idx + m, base + idx] = float(int(w_core[f, m, n, 0]))
    i16 = np.arange(16)
    for t in range(2):
        for n in range(3):
            base = F_PER_CORE * 3 * LHST_MAIN + (t * 3 + n) * LHST_REM
            for g in range(4):
                f = 4 * t + g
                b_f = int(b_core[f])
                bh, bl = b_f >> 3, b_f - 8 * (b_f >> 3)
                col = base + 16 * g
                if n == 0:
                    out[0, col : col + 16] = float(8 * bh)
                    out[1, col : col + 16] = float(bl)
                for m in range(3):
                    out[2 + 18 * g + i16 + m, col + i16] = float(int(w_core[f, m, n, 0]))
    return out.astype(ml_dtypes.bfloat16)


_PROGRAM_CACHE = {}


def _build_program():
    import concourse.bass as bass
    import concourse.tile as tile
    from concourse import bacc, mybir

    nc = bacc.Bacc(
        "TRN2", target_bir_lowering=False, debug=False, num_devices=N_CORES
    )
    dt = mybir.dt
    Alu = mybir.AluOpType
    Act = mybir.ActivationFunctionType

    x_d = nc.dram_tensor(
        "x", [F_PER_CORE, H_IN, FREE_IN], dt.int8, kind="ExternalInput"
    ).ap()
    lhsT_d = nc.dram_tensor(
        "lhsT", [128, LHST_COLS], dt.bfloat16, kind="ExternalInput"
    ).ap()
    ones_d = nc.dram_tensor(
        "ones2", [2, XT_COLS], dt.int8, kind="ExternalInput"
    ).ap()
    y_d = nc.dram_tensor(
        "y", [F_PER_CORE, H_OUT, WD_OUT], dt.int8, kind="ExternalOutput"
    ).ap()

    with tile.TileContext(nc) as tc:
        with (
            tc.tile_pool(name="const", bufs=1) as const_pool,
            tc.tile_pool(name="xi8", bufs=2) as xi_pool,
            tc.tile_pool(name="xbf", bufs=2) as xb_pool,
            tc.tile_pool(name="xrem", bufs=2) as xr_pool,
            tc.tile_pool(name="psum", bufs=2, space="PSUM") as psum_pool,
            tc.tile_pool(name="otile", bufs=4) as o_pool,
        ):
            lhsT_t = const_pool.tile([128, LHST_COLS], dt.bfloat16)
            nc.sync.dma_start(lhsT_t[:], lhsT_d[:])

            def requant_store(ps, m_hi, widx, dst_ap):
                """One-op requant of ps[0:m_hi, :] -> int8 tile -> DMA out."""
                ot = o_pool.tile([128, WD_OUT], dt.int8)
                if widx % 2 == 0:
                    nc.scalar.activation(
                        ot[0:m_hi, :], ps[0:m_hi, :], Act.Copy,
                        bias=B0, scale=float(CPF),
                    )
                else:
                    nc.vector.tensor_scalar(
                        ot[0:m_hi, :], ps[0:m_hi, :], float(CPF), B0,
                        Alu.mult, Alu.add,
                    )
                if widx % 2 == 0:
                    nc.gpsimd.dma_start(dst_ap, ot[0:m_hi, :])
                else:
                    nc.sync.dma_start(dst_ap, ot[0:m_hi, :])

            widx = 0
            for wi, (r0, m_r) in enumerate(MAIN_WINDOWS):
                xi = xi_pool.tile([128, XT_COLS], dt.int8)
                nc.sync.dma_start(xi[0:2, :], ones_d[:])
                # one plain SWDGE int8 DMA for all 8 channels of this window:
                # dst partition 2+p, free f*2056 + j  <-  x[f, r0+p, j]
                nc.gpsimd.dma_start(
                    xi[2 : 2 + m_r + 2, :],
                    x_d[:, r0 : r0 + m_r + 2, :].transpose([1, 0, 2]),
                )
                # whole-tile on-chip cast int8 -> bf16 (incl. the ones rows)
                xt = xb_pool.tile([128, XT_COLS], dt.bfloat16)
                if wi % 2 == 0:
                    nc.scalar.activation(xt[:], xi[:], Act.Copy)
                else:
                    nc.vector.tensor_copy(xt[:], xi[:])
                for f in range(F_PER_CORE):
                    ps = psum_pool.tile([128, WD_OUT], dt.float32)
                    for n in range(3):
                        base = (f * 3 + n) * LHST_MAIN
                        xoff = f * X_BLOCK + 4 * n
                        for c in range(4):
                            nc.tensor.matmul(
                                ps[:, c * N_CHUNK : (c + 1) * N_CHUNK],
                                lhsT_t[:, base : base + LHST_MAIN],
                                xt[:, xoff + c * N_CHUNK : xoff + c * N_CHUNK + N_CHUNK],
                                start=(n == 0),
                                stop=(n == 2),
                                skip_group_check=True,
                            )
                    requant_store(ps, m_r, widx, y_d[f, r0 : r0 + m_r, :])
                    widx += 1

            # remainder: output rows 496..511, 4 channels per matmul group
            for t in range(2):
                xri = xr_pool.tile([74, FREE_IN], dt.int8)
                nc.sync.dma_start(xri[0:2, :], ones_d[0:2, 0:FREE_IN])
                # dst partition 2 + 18g + p  <-  x[4t+g, 496+p, :]
                nc.gpsimd.dma_start(
                    xri[2:74, :],
                    x_d[4 * t : 4 * t + 4, R0_REM : R0_REM + 18, :],
                )
                xr = xr_pool.tile([74, FREE_IN], dt.bfloat16)
                if t == 0:
                    nc.scalar.activation(xr[:], xri[:], Act.Copy)
                else:
                    nc.vector.tensor_copy(xr[:], xri[:])
                ps = psum_pool.tile([128, WD_OUT], dt.float32)
                for n in range(3):
                    base = F_PER_CORE * 3 * LHST_MAIN + (t * 3 + n) * LHST_REM
                    for c in range(4):
                        nc.tensor.matmul(
                            ps[0:LHST_REM, c * N_CHUNK : (c + 1) * N_CHUNK],
                            lhsT_t[0:74, base : base + LHST_REM],
                            xr[:, 4 * n + c * N_CHUNK : 4 * n + c * N_CHUNK + N_CHUNK],
                            start=(n == 0),
                            stop=(n == 2),
                            skip_group_check=True,
                        )
                # one out-DMA: dst y[4t+g, 496+i, :] <- ot[16g+i, :]
                requant_store(
                    ps, LHST_REM, widx,
                    y_d[4 * t : 4 * t + 4, R0_REM:H_OUT, :],
                )
                widx += 1

    nc.compile()
    return nc


def make_in_maps(x: np.ndarray, w: np.ndarray, b: np.ndarray):
    ones2 = np.ones((2, XT_COLS), dtype=np.int8)
    in_maps = []
    for core in range(N_CORES):
        lo = core * F_PER_CORE
        hi = lo + F_PER_CORE
        x_shard = np.ascontiguousarray(x[lo:hi]).reshape(F_PER_CORE, H_IN, FREE_IN)
        lhsT = _build_lhsT(w[lo:hi], b[lo:hi])
        in_maps.append({"x": x_shard, "lhsT": lhsT, "ones2": ones2})
    return in_maps


def kernel(x: np.ndarray, w: np.ndarray, b: np.ndarray) -> np.ndarray:
    """x: int8 [64, 514, 514, 4]; w: int8 [64, 3, 3, 1]; b: int32 [64].

    Returns int8 [64, 512, 512, 4].
    """
    from concourse.bass_utils import run_bass_kernel_spmd

    if "nc" not in _PROGRAM_CACHE:
        _PROGRAM_CACHE["nc"] = _build_program()
    nc = _PROGRAM_CACHE["nc"]

    F = x.shape[0]
    assert F == N_CORES * F_PER_CORE

    res = run_bass_kernel_spmd(
        nc, make_in_maps(x, w, b), core_ids=list(range(N_CORES))
    )

    out = np.empty((F, H_OUT, 512, D), dtype=np.int8)
    for core in range(N_CORES):
        lo = core * F_PER_CORE
        y = res.results[core]["y"]  # [8, 512, 2048] int8
        out[lo : lo + F_PER_CORE] = y.reshape(F_PER_CORE, H_OUT, 512, D)
    return out


# revision 7
# speedup vs baseline: 1.0678x; 1.0108x over previous
"""Trainium2 Bass kernel for nn_Conv2D_80796924772741.

Depthwise (grouped, F=64) 3x3 valid conv over [F, 514, 514, 4] int8 with
per-channel int8 weights + int32 bias, followed by exact fixed-point requant
  res = (acc * 19920 + 2^21) >> 22 ;  out = clip(res - 5, -128, 127) int8
(reduced_mantissa 19920 = 1245 * 16 -> res = (acc*1245 + 2^17) >> 18).

Sharding: F=64 split across 8 NeuronCores (8 channels each), embarrassingly
parallel.

Per-core structure (v3):
 - x is loaded window-at-a-time for ALL 8 channels in ONE plain int8 SWDGE
   DMA (the int8->bf16 cast-during-DMA path measured only ~14 GB/s/engine,
   so the cast is done on-chip instead): int8 tile [128, 8*2056], partitions
   0..1 all-ones (int8 ones input; they drive the bias rows), partitions
   2..127 the 126 window rows, channel f at free block f*2056.
 - One whole-tile cast op int8 -> bf16 per window, alternating ACT / DVE.
 - PE: per (channel, window) a [128-row, 2048] PSUM mega-tile (4 banks)
   accumulated by 12 matmuls: Toeplitz-band stationary lhsT (3 H-taps as
   band diagonals, K = 128 incl. 2 bias rows), 3 W-taps as rhs free offset
   +4n, 4 x 512-column chunks.  Bands padded to 128 columns (FWL).
 - Last 16 output rows: 4 channels packed per matmul via block-diagonal
   bands ([74, 64] with shared ones rows), 2 groups.
 - Requant in ONE op per (channel, window) reading the 4-bank PSUM AP:
     out = sat_i8(rne(A * C + B0)),  C = f32(1245/2^18), B0 = f32(-5+4*2^-21)
   alternating ACT / DVE.  Verified bit-exact offline over every
   A in [-147304, 147304] and on-hardware on both engines.
 - Output DMAs alternate SWDGE (gpsimd) / HWDGE (sync) so the y-writes
   spread across all 16 SDMA engines (HWDGE alone was observed to use 4).
"""

import numpy as np
import ml_dtypes

F_PER_CORE = 8
H_IN = 514
W_IN = 514
D = 4
H_OUT = 512
WD_OUT = 2048  # 512 * 4
FREE_IN = W_IN * D  # 2056
N_CHUNK = 512
N_CORES = 8
M_WIN = 124
MAIN_WINDOWS = [(0, 124), (124, 124), (248, 124), (372, 124)]
R0_REM = 496  # remainder: output rows 496..511, 16 per channel
X_BLOCK = FREE_IN
XT_COLS = F_PER_CORE * X_BLOCK  # 16448

LHST_MAIN = 128  # padded band width (FWL) per (f, n) block
LHST_REM = 64  # block-diag remainder width per (group, n) block
LHST_COLS = F_PER_CORE * 3 * LHST_MAIN + 2 * 3 * LHST_REM  # 3456

CPF = np.float32(1245.0 / 2.0**18)
B0 = float(np.float32(-5.0 + 4 * 2.0**-21))


def _build_lhsT(w_core: np.ndarray, b_core: np.ndarray) -> np.ndarray:
    """[128, LHST_COLS] bf16 stationary weights (see module docstring)."""
    out = np.zeros((128, LHST_COLS), dtype=np.float32)
    idx = np.arange(M_WIN)
    for f in range(F_PER_CORE):
        b_f = int(b_core[f])
        bh, bl = b_f >> 3, b_f - 8 * (b_f >> 3)
        for n in range(3):
            base = (f * 3 + n) * LHST_MAIN
            if n == 0:
                out[0, base : base + M_WIN] = float(8 * bh)
                out[1, base : base + M_WIN] = float(bl)
            for m in range(3):
                out[2 + idx + m, base + idx] = float(int(w_core[f, m, n, 0]))
    i16 = np.arange(16)
    for t in range(2):
        for n in range(3):
            base = F_PER_CORE * 3 * LHST_MAIN + (t * 3 + n) * LHST_REM
            for g in range(4):
                f = 4 * t + g
                b_f = int(b_core[f])
                bh, bl = b_f >> 3, b_f - 8 * (b_f >> 3)
                col = base + 16 * g
                if n == 0:
                    out[0, col : col + 16] = float(8 * bh)
                    out[1, col : col + 16] = float(bl)
                for m in range(3):
                    out[2 + 18 * g + i16 + m, col + i16] = float(int(w_core[f, m, n, 0]))
    return out.astype(ml_dtypes.bfloat16)


_PROGRAM_CACHE = {}


def _build_program():
    import concourse.bass as bass
    import concourse.tile as tile
    from concourse import bacc, mybir

    nc = bacc.Bacc(
        "TRN2", target_bir_lowering=False, debug=False, num_devices=N_CORES
    )
    dt = mybir.dt
    Alu = mybir.AluOpType
    Act = mybir.ActivationFunctionType

    x_d = nc.dram_tensor(
        "x", [F_PER_CORE, H_IN, FREE_IN], dt.int8, kind="ExternalInput"
    ).ap()
    lhsT_d = nc.dram_tensor(
        "lhsT", [128, LHST_COLS], dt.bfloat16, kind="ExternalInput"
    ).ap()
    ones_d = nc.dram_tensor(
        "ones2", [2, XT_COLS], dt.int8, kind="ExternalInput"
    ).ap()
    y_d = nc.dram_tensor(
        "y", [F_PER_CORE, H_OUT, WD_OUT], dt.int8, kind="ExternalOutput"
    ).ap()

    with tile.TileContext(nc) as tc:
        with (
            tc.tile_pool(name="const", bufs=1) as const_pool,
            tc.tile_pool(name="xi8", bufs=3) as xi_pool,
            tc.tile_pool(name="xbf", bufs=3) as xb_pool,
            tc.tile_pool(name="xrem", bufs=2) as xr_pool,
            tc.tile_pool(name="psum", bufs=2, space="PSUM") as psum_pool,
            tc.tile_pool(name="otile", bufs=3) as o_pool,
        ):
            lhsT_t = const_pool.tile([128, LHST_COLS], dt.bfloat16)
            nc.sync.dma_start(lhsT_t[:], lhsT_d[:])

            def requant_store(ps, m_hi, widx, dst_ap):
                """One-op requant of ps[0:m_hi, :] -> int8 tile -> DMA out."""
                ot = o_pool.tile([128, WD_OUT], dt.int8)
                if widx % 3 != 2:
                    nc.scalar.activation(
                        ot[0:m_hi, :], ps[0:m_hi, :], Act.Copy,
                        bias=B0, scale=float(CPF),
                    )
                else:
                    nc.vector.tensor_scalar(
                        ot[0:m_hi, :], ps[0:m_hi, :], float(CPF), B0,
                        Alu.mult, Alu.add,
                    )
                if widx % 2 == 0:
                    nc.gpsimd.dma_start(dst_ap, ot[0:m_hi, :])
                else:
                    nc.sync.dma_start(dst_ap, ot[0:m_hi, :])

            HALF = XT_COLS // 2  # 4 channels per half-tile
            widx = 0
            for wi, (r0, m_r) in enumerate(MAIN_WINDOWS):
                xi = xi_pool.tile([128, XT_COLS], dt.int8)
                xt = xb_pool.tile([128, XT_COLS], dt.bfloat16)
                nc.sync.dma_start(xi[0:2, :], ones_d[:])
                # per-half plain int8 SWDGE + DVE cast for finer pipelining:
                # dst partition 2+p, free f*2056 + j  <-  x[f, r0+p, j]
                for h in range(2):
                    nc.gpsimd.dma_start(
                        xi[2 : 2 + m_r + 2, h * HALF : (h + 1) * HALF],
                        x_d[4 * h : 4 * h + 4, r0 : r0 + m_r + 2, :].transpose(
                            [1, 0, 2]
                        ),
                    )
                    nc.vector.tensor_copy(
                        xt[:, h * HALF : (h + 1) * HALF],
                        xi[:, h * HALF : (h + 1) * HALF],
                    )
                for f in range(F_PER_CORE):
                    ps = psum_pool.tile([128, WD_OUT], dt.float32)
                    for n in range(3):
                        base = (f * 3 + n) * LHST_MAIN
                        xoff = f * X_BLOCK + 4 * n
                        for c in range(4):
                            nc.tensor.matmul(
                                ps[:, c * N_CHUNK : (c + 1) * N_CHUNK],
                                lhsT_t[:, base : base + LHST_MAIN],
                                xt[:, xoff + c * N_CHUNK : xoff + c * N_CHUNK + N_CHUNK],
                                start=(n == 0),
                                stop=(n == 2),
                                skip_group_check=True,
                            )
                    requant_store(ps, m_r, widx, y_d[f, r0 : r0 + m_r, :])
                    widx += 1

            # remainder: output rows 496..511, 4 channels per matmul group
            for t in range(2):
                xri = xr_pool.tile([74, FREE_IN], dt.int8)
                nc.sync.dma_start(xri[0:2, :], ones_d[0:2, 0:FREE_IN])
                # dst partition 2 + 18g + p  <-  x[4t+g, 496+p, :]
                nc.gpsimd.dma_start(
                    xri[2:74, :],
                    x_d[4 * t : 4 * t + 4, R0_REM : R0_REM + 18, :],
                )
                xr = xr_pool.tile([74, FREE_IN], dt.bfloat16)
                if t == 0:
                    nc.scalar.activation(xr[:], xri[:], Act.Copy)
                else:
                    nc.vector.tensor_copy(xr[:], xri[:])
                ps = psum_pool.tile([128, WD_OUT], dt.float32)
                for n in range(3):
                    base = F_PER_CORE * 3 * LHST_MAIN + (t * 3 + n) * LHST_REM
                    for c in range(4):
                        nc.tensor.matmul(
                            ps[0:LHST_REM, c * N_CHUNK : (c + 1) * N_CHUNK],
                            lhsT_t[0:74, base : base + LHST_REM],
                            xr[:, 4 * n + c * N_CHUNK : 4 * n + c * N_CHUNK + N_CHUNK],
                            start=(n == 0),
                            stop=(n == 2),
                            skip_group_check=True,
                        )
                # one out-DMA: dst y[4t+g, 496+i, :] <- ot[16g+i, :]
                requant_store(
                    ps, LHST_REM, widx,
                    y_d[4 * t : 4 * t + 4, R0_REM:H_OUT, :],
                )
                widx += 1

    nc.compile()
    return nc


def make_in_maps(x: np.ndarray, w: np.ndarray, b: np.ndarray):
    ones2 = np.ones((2, XT_COLS), dtype=np.int8)
    in_maps = []
    for core in range(N_CORES):
        lo = core * F_PER_CORE
        hi = lo + F_PER_CORE
        x_shard = np.ascontiguousarray(x[lo:hi]).reshape(F_PER_CORE, H_IN, FREE_IN)
        lhsT = _build_lhsT(w[lo:hi], b[lo:hi])
        in_maps.append({"x": x_shard, "lhsT": lhsT, "ones2": ones2})
    return in_maps


def kernel(x: np.ndarray, w: np.ndarray, b: np.ndarray) -> np.ndarray:
    """x: int8 [64, 514, 514, 4]; w: int8 [64, 3, 3, 1]; b: int32 [64].

    Returns int8 [64, 512, 512, 4].
    """
    from concourse.bass_utils import run_bass_kernel_spmd

    if "nc" not in _PROGRAM_CACHE:
        _PROGRAM_CACHE["nc"] = _build_program()
    nc = _PROGRAM_CACHE["nc"]

    F = x.shape[0]
    assert F == N_CORES * F_PER_CORE

    res = run_bass_kernel_spmd(
        nc, make_in_maps(x, w, b), core_ids=list(range(N_CORES))
    )

    out = np.empty((F, H_OUT, 512, D), dtype=np.int8)
    for core in range(N_CORES):
        lo = core * F_PER_CORE
        y = res.results[core]["y"]  # [8, 512, 2048] int8
        out[lo : lo + F_PER_CORE] = y.reshape(F_PER_CORE, H_OUT, 512, D)
    return out


# revision 11
# speedup vs baseline: 1.2452x; 1.1661x over previous
"""Trainium2 Bass kernel for nn_Conv2D_80796924772741.

Depthwise (grouped, F=64) 3x3 valid conv over [F, 514, 514, 4] int8 with
per-channel int8 weights + int32 bias, followed by exact fixed-point requant
  res = (acc * 19920 + 2^21) >> 22 ;  out = clip(res - 5, -128, 127) int8
(reduced_mantissa 19920 = 1245 * 16 -> res = (acc*1245 + 2^17) >> 18).

Sharding: F=64 split across 8 NeuronCores (8 channels each), embarrassingly
parallel.

Per-core structure (v3):
 - x is loaded window-at-a-time for ALL 8 channels in ONE plain int8 SWDGE
   DMA (the int8->bf16 cast-during-DMA path measured only ~14 GB/s/engine,
   so the cast is done on-chip instead): int8 tile [128, 8*2056], partitions
   0..1 all-ones (int8 ones input; they drive the bias rows), partitions
   2..127 the 126 window rows, channel f at free block f*2056.
 - One whole-tile cast op int8 -> bf16 per window, alternating ACT / DVE.
 - PE: per (channel, window) a [128-row, 2048] PSUM mega-tile (4 banks)
   accumulated by 12 matmuls: Toeplitz-band stationary lhsT (3 H-taps as
   band diagonals, K = 128 incl. 2 bias rows), 3 W-taps as rhs free offset
   +4n, 4 x 512-column chunks.  Bands padded to 128 columns (FWL).
 - Last 16 output rows: 4 channels packed per matmul via block-diagonal
   bands ([74, 64] with shared ones rows), 2 groups.
 - Requant in ONE op per (channel, window) reading the 4-bank PSUM AP:
     out = sat_i8(rne(A * C + B0)),  C = f32(1245/2^18), B0 = f32(-5+4*2^-21)
   alternating ACT / DVE.  Verified bit-exact offline over every
   A in [-147304, 147304] and on-hardware on both engines.
 - Output DMAs alternate SWDGE (gpsimd) / HWDGE (sync) so the y-writes
   spread across all 16 SDMA engines (HWDGE alone was observed to use 4).
"""

import numpy as np
import ml_dtypes

F_PER_CORE = 8
H_IN = 514
W_IN = 514
D = 4
H_OUT = 512
WD_OUT = 2048  # 512 * 4
FREE_IN = W_IN * D  # 2056
N_CHUNK = 512
N_CORES = 8
M_WIN = 124
MAIN_WINDOWS = [(0, 124), (124, 124), (248, 124), (372, 124)]
R0_REM = 496  # remainder: output rows 496..511, 16 per channel
X_BLOCK = FREE_IN
XT_COLS = F_PER_CORE * X_BLOCK  # 16448

LHST_MAIN = 128  # padded band width (FWL) per (f, n) block
LHST_REM = 64  # block-diag remainder width per (group, n) block
LHST_COLS = F_PER_CORE * 3 * LHST_MAIN + 2 * 3 * LHST_REM  # 3456

CPF = np.float32(1245.0 / 2.0**18)
B0 = float(np.float32(-5.0 + 4 * 2.0**-21))


def _build_lhsT(w_core: np.ndarray, b_core: np.ndarray) -> np.ndarray:
    """[128, LHST_COLS] bf16 stationary weights (see module docstring)."""
    out = np.zeros((128, LHST_COLS), dtype=np.float32)
    idx = np.arange(M_WIN)
    for f in range(F_PER_CORE):
        b_f = int(b_core[f])
        bh, bl = b_f >> 3, b_f - 8 * (b_f >> 3)
        for n in range(3):
            base = (f * 3 + n) * LHST_MAIN
            if n == 0:
                out[0, base : base + M_WIN] = float(8 * bh)
                out[1, base : base + M_WIN] = float(bl)
            for m in range(3):
                out[2 + idx + m, base + idx] = float(int(w_core[f, m, n, 0]))
    i16 = np.arange(16)
    for t in range(2):
        for n in range(3):
            base = F_PER_CORE * 3 * LHST_MAIN + (t * 3 + n) * LHST_REM
            for g in range(4):
                f = 4 * t + g
                b_f = int(b_core[f])
                bh, bl = b_f >> 3, b_f - 8 * (b_f >> 3)
                col = base + 16 * g
                if n == 0:
                    out[0, col : col + 16] = float(8 * bh)
                    out[1, col : col + 16] = float(bl)
                for m in range(3):
                    out[2 + 18 * g + i16 + m, col + i16] = float(int(w_core[f, m, n, 0]))
    return out.astype(ml_dtypes.bfloat16)


_PROGRAM_CACHE = {}


def _build_program():
    import concourse.bass as bass
    import concourse.tile as tile
    from concourse import bacc, mybir

    nc = bacc.Bacc(
        "TRN2", target_bir_lowering=False, debug=False, num_devices=N_CORES
    )
    dt = mybir.dt
    Alu = mybir.AluOpType
    Act = mybir.ActivationFunctionType

    x_d = nc.dram_tensor(
        "x", [F_PER_CORE, H_IN, FREE_IN], dt.int8, kind="ExternalInput"
    ).ap()
    lhsT_d = nc.dram_tensor(
        "lhsT", [128, LHST_COLS], dt.bfloat16, kind="ExternalInput"
    ).ap()
    ones_d = nc.dram_tensor(
        "ones2", [2, XT_COLS], dt.int8, kind="ExternalInput"
    ).ap()
    y_d = nc.dram_tensor(
        "y", [F_PER_CORE, H_OUT, WD_OUT], dt.int8, kind="ExternalOutput"
    ).ap()

    with tile.TileContext(nc) as tc:
        with (
            tc.tile_pool(name="const", bufs=1) as const_pool,
            tc.tile_pool(name="xi8", bufs=3) as xi_pool,
            tc.tile_pool(name="xbf", bufs=2) as xb_pool,
            tc.tile_pool(name="xrem", bufs=2) as xr_pool,
            tc.tile_pool(name="psum", bufs=2, space="PSUM") as psum_pool,
            tc.tile_pool(name="otile", bufs=6) as o_pool,
        ):
            lhsT_t = const_pool.tile([128, LHST_COLS], dt.bfloat16)
            nc.sync.dma_start(lhsT_t[:], lhsT_d[:])

            def requant_store(ps, m_hi, widx, dst_ap):
                """One-op requant of ps[0:m_hi, :] -> int8 tile -> DMA out.

                All requants on ACT (casts own the DVE queue; mixing them
                head-of-line blocks the next window's cast behind a stalled
                requant in the strict-FIFO engine queue).
                """
                ot = o_pool.tile([128, WD_OUT], dt.int8)
                nc.scalar.activation(
                    ot[0:m_hi, :], ps[0:m_hi, :], Act.Copy,
                    bias=B0, scale=float(CPF),
                )
                if widx % 2 == 0:
                    nc.gpsimd.dma_start(dst_ap, ot[0:m_hi, :])
                else:
                    nc.sync.dma_start(dst_ap, ot[0:m_hi, :])

            def emit_remainder(t, widx):
                """Output rows 496..511, 4 channels per matmul group."""
                xri = xr_pool.tile([74, FREE_IN], dt.int8)
                nc.sync.dma_start(xri[0:2, :], ones_d[0:2, 0:FREE_IN])
                # dst partition 2 + 18g + p  <-  x[4t+g, 496+p, :]
                nc.gpsimd.dma_start(
                    xri[2:74, :],
                    x_d[4 * t : 4 * t + 4, R0_REM : R0_REM + 18, :],
                )
                xr = xr_pool.tile([74, FREE_IN], dt.bfloat16)
                nc.vector.tensor_copy(xr[:], xri[:])
                ps = psum_pool.tile([128, WD_OUT], dt.float32)
                for n in range(3):
                    base = F_PER_CORE * 3 * LHST_MAIN + (t * 3 + n) * LHST_REM
                    for c in range(4):
                        nc.tensor.matmul(
                            ps[0:LHST_REM, c * N_CHUNK : (c + 1) * N_CHUNK],
                            lhsT_t[0:74, base : base + LHST_REM],
                            xr[:, 4 * n + c * N_CHUNK : 4 * n + c * N_CHUNK + N_CHUNK],
                            start=(n == 0),
                            stop=(n == 2),
                            skip_group_check=True,
                        )
                # one out-DMA: dst y[4t+g, 496+i, :] <- ot[16g+i, :]
                requant_store(
                    ps, LHST_REM, widx,
                    y_d[4 * t : 4 * t + 4, R0_REM:H_OUT, :],
                )

            HALF = XT_COLS // 2  # 4 channels per half-tile
            widx = 0
            for wi, (r0, m_r) in enumerate(MAIN_WINDOWS):
                xi = xi_pool.tile([128, XT_COLS], dt.int8)
                xt = xb_pool.tile([128, XT_COLS], dt.bfloat16)
                nc.sync.dma_start(xi[0:2, :], ones_d[:])
                # per-half plain int8 SWDGE + DVE cast for finer pipelining:
                # dst partition 2+p, free f*2056 + j  <-  x[f, r0+p, j]
                for h in range(2):
                    nc.gpsimd.dma_start(
                        xi[2 : 2 + m_r + 2, h * HALF : (h + 1) * HALF],
                        x_d[4 * h : 4 * h + 4, r0 : r0 + m_r + 2, :].transpose(
                            [1, 0, 2]
                        ),
                    )
                    nc.vector.tensor_copy(
                        xt[:, h * HALF : (h + 1) * HALF],
                        xi[:, h * HALF : (h + 1) * HALF],
                    )
                for f in range(F_PER_CORE):
                    ps = psum_pool.tile([128, WD_OUT], dt.float32)
                    for n in range(3):
                        base = (f * 3 + n) * LHST_MAIN
                        xoff = f * X_BLOCK + 4 * n
                        for c in range(4):
                            nc.tensor.matmul(
                                ps[:, c * N_CHUNK : (c + 1) * N_CHUNK],
                                lhsT_t[:, base : base + LHST_MAIN],
                                xt[:, xoff + c * N_CHUNK : xoff + c * N_CHUNK + N_CHUNK],
                                start=(n == 0),
                                stop=(n == 2),
                                skip_group_check=True,
                            )
                    requant_store(ps, m_r, widx, y_d[f, r0 : r0 + m_r, :])
                    widx += 1
                # interleave the two remainder groups mid-stream so their
                # matmuls fill PE bubbles instead of forming a tail
                if wi == 1 or wi == 2:
                    emit_remainder(wi - 1, widx)
                    widx += 1

    nc.compile()
    return nc


def make_in_maps(x: np.ndarray, w: np.ndarray, b: np.ndarray):
    ones2 = np.ones((2, XT_COLS), dtype=np.int8)
    in_maps = []
    for core in range(N_CORES):
        lo = core * F_PER_CORE
        hi = lo + F_PER_CORE
        x_shard = np.ascontiguousarray(x[lo:hi]).reshape(F_PER_CORE, H_IN, FREE_IN)
        lhsT = _build_lhsT(w[lo:hi], b[lo:hi])
        in_maps.append({"x": x_shard, "lhsT": lhsT, "ones2": ones2})
    return in_maps


def kernel(x: np.ndarray, w: np.ndarray, b: np.ndarray) -> np.ndarray:
    """x: int8 [64, 514, 514, 4]; w: int8 [64, 3, 3, 1]; b: int32 [64].

    Returns int8 [64, 512, 512, 4].
    """
    from concourse.bass_utils import run_bass_kernel_spmd

    if "nc" not in _PROGRAM_CACHE:
        _PROGRAM_CACHE["nc"] = _build_program()
    nc = _PROGRAM_CACHE["nc"]

    F = x.shape[0]
    assert F == N_CORES * F_PER_CORE

    res = run_bass_kernel_spmd(
        nc, make_in_maps(x, w, b), core_ids=list(range(N_CORES))
    )

    out = np.empty((F, H_OUT, 512, D), dtype=np.int8)
    for core in range(N_CORES):
        lo = core * F_PER_CORE
        y = res.results[core]["y"]  # [8, 512, 2048] int8
        out[lo : lo + F_PER_CORE] = y.reshape(F_PER_CORE, H_OUT, 512, D)
    return out


# revision 15
# speedup vs baseline: 1.3089x; 1.0512x over previous
"""Trainium2 Bass kernel for nn_Conv2D_80796924772741.

Depthwise (grouped, F=64) 3x3 valid conv over [F, 514, 514, 4] int8 with
per-channel int8 weights + int32 bias, followed by exact fixed-point requant
  res = (acc * 19920 + 2^21) >> 22 ;  out = clip(res - 5, -128, 127) int8
(reduced_mantissa 19920 = 1245 * 16 -> res = (acc*1245 + 2^17) >> 18).

Sharding: F=64 split across 8 NeuronCores (8 channels each), embarrassingly
parallel.

Per-core structure (v3):
 - x is loaded window-at-a-time for ALL 8 channels in ONE plain int8 SWDGE
   DMA (the int8->bf16 cast-during-DMA path measured only ~14 GB/s/engine,
   so the cast is done on-chip instead): int8 tile [128, 8*2056], partitions
   0..1 all-ones (int8 ones input; they drive the bias rows), partitions
   2..127 the 126 window rows, channel f at free block f*2056.
 - One whole-tile cast op int8 -> bf16 per window, alternating ACT / DVE.
 - PE: per (channel, window) a [128-row, 2048] PSUM mega-tile (4 banks)
   accumulated by 12 matmuls: Toeplitz-band stationary lhsT (3 H-taps as
   band diagonals, K = 128 incl. 2 bias rows), 3 W-taps as rhs free offset
   +4n, 4 x 512-column chunks.  Bands padded to 128 columns (FWL).
 - Last 16 output rows: 4 channels packed per matmul via block-diagonal
   bands ([74, 64] with shared ones rows), 2 groups.
 - Requant in ONE op per (channel, window) reading the 4-bank PSUM AP:
     out = sat_i8(rne(A * C + B0)),  C = f32(1245/2^18), B0 = f32(-5+4*2^-21)
   alternating ACT / DVE.  Verified bit-exact offline over every
   A in [-147304, 147304] and on-hardware on both engines.
 - Output DMAs alternate SWDGE (gpsimd) / HWDGE (sync) so the y-writes
   spread across all 16 SDMA engines (HWDGE alone was observed to use 4).
"""

import numpy as np
import ml_dtypes

F_PER_CORE = 8
H_IN = 514
W_IN = 514
D = 4
H_OUT = 512
WD_OUT = 2048  # 512 * 4
FREE_IN = W_IN * D  # 2056
N_CHUNK = 512
N_CORES = 8
M_WIN = 124
MAIN_WINDOWS = [(0, 124), (124, 124), (248, 124), (372, 124)]
R0_REM = 496  # remainder: output rows 496..511, 16 per channel
X_BLOCK = FREE_IN
XT_COLS = F_PER_CORE * X_BLOCK  # 16448

LHST_MAIN = 128  # padded band width (FWL) per (f, n) block
LHST_REM = 64  # block-diag remainder width per (group, n) block
LHST_COLS = F_PER_CORE * 3 * LHST_MAIN + 2 * 3 * LHST_REM  # 3456

CPF = np.float32(1245.0 / 2.0**18)
B0 = float(np.float32(-5.0 + 4 * 2.0**-21))


def _build_lhsT(w_core: np.ndarray, b_core: np.ndarray) -> np.ndarray:
    """[128, LHST_COLS] bf16 stationary weights (see module docstring)."""
    out = np.zeros((128, LHST_COLS), dtype=np.float32)
    idx = np.arange(M_WIN)
    for f in range(F_PER_CORE):
        b_f = int(b_core[f])
        bh, bl = b_f >> 3, b_f - 8 * (b_f >> 3)
        for n in range(3):
            base = (f * 3 + n) * LHST_MAIN
            if n == 0:
                out[0, base : base + M_WIN] = float(8 * bh)
                out[1, base : base + M_WIN] = float(bl)
            for m in range(3):
                out[2 + idx + m, base + idx] = float(int(w_core[f, m, n, 0]))
    i16 = np.arange(16)
    for t in range(2):
        for n in range(3):
            base = F_PER_CORE * 3 * LHST_MAIN + (t * 3 + n) * LHST_REM
            for g in range(4):
                f = 4 * t + g
                b_f = int(b_core[f])
                bh, bl = b_f >> 3, b_f - 8 * (b_f >> 3)
                col = base + 16 * g
                if n == 0:
                    out[0, col : col + 16] = float(8 * bh)
                    out[1, col : col + 16] = float(bl)
                for m in range(3):
                    out[2 + 18 * g + i16 + m, col + i16] = float(int(w_core[f, m, n, 0]))
    return out.astype(ml_dtypes.bfloat16)


_PROGRAM_CACHE = {}


def _build_program():
    import concourse.bass as bass
    import concourse.tile as tile
    from concourse import bacc, mybir

    nc = bacc.Bacc(
        "TRN2", target_bir_lowering=False, debug=False, num_devices=N_CORES
    )
    dt = mybir.dt
    Alu = mybir.AluOpType
    Act = mybir.ActivationFunctionType

    x_d = nc.dram_tensor(
        "x", [F_PER_CORE, H_IN, FREE_IN], dt.int8, kind="ExternalInput"
    ).ap()
    lhsT_d = nc.dram_tensor(
        "lhsT", [128, LHST_COLS], dt.bfloat16, kind="ExternalInput"
    ).ap()
    ones_d = nc.dram_tensor(
        "ones2", [2, XT_COLS], dt.int8, kind="ExternalInput"
    ).ap()
    y_d = nc.dram_tensor(
        "y", [F_PER_CORE, H_OUT, WD_OUT], dt.int8, kind="ExternalOutput"
    ).ap()

    with tile.TileContext(nc) as tc:
        with (
            tc.tile_pool(name="const", bufs=1) as const_pool,
            tc.tile_pool(name="xi8", bufs=3) as xi_pool,
            tc.tile_pool(name="xbf", bufs=3) as xb_pool,
            tc.tile_pool(name="xremi", bufs=2) as xri_pool,
            tc.tile_pool(name="xremb", bufs=2) as xrb_pool,
            tc.tile_pool(name="psum", bufs=2, space="PSUM") as psum_pool,
            tc.tile_pool(name="otile", bufs=6) as o_pool,
        ):
            lhsT_t = const_pool.tile([128, LHST_COLS], dt.bfloat16)
            nc.sync.dma_start(lhsT_t[:], lhsT_d[:])

            def requant_store(ps, m_hi, widx, dst_ap, dve=False):
                """One-op requant of ps[0:m_hi, :] -> int8 tile -> DMA out.

                Requants run on ACT (casts own the DVE queue; mixing them
                head-of-line blocks the next window's cast behind a stalled
                requant in the strict-FIFO engine queue).  The final window
                splits ACT/DVE (dve=True) to halve the drain tail.
                """
                ot = o_pool.tile([128, WD_OUT], dt.int8)
                if dve:
                    nc.vector.tensor_scalar(
                        ot[0:m_hi, :], ps[0:m_hi, :], float(CPF), B0,
                        Alu.mult, Alu.add,
                    )
                else:
                    nc.scalar.activation(
                        ot[0:m_hi, :], ps[0:m_hi, :], Act.Copy,
                        bias=B0, scale=float(CPF),
                    )
                if widx % 2 == 0:
                    nc.gpsimd.dma_start(dst_ap, ot[0:m_hi, :])
                else:
                    nc.sync.dma_start(dst_ap, ot[0:m_hi, :])

            rem_bf = {}

            def emit_remainder_load(t):
                """Load + cast for remainder rows (496..511, channel group t)."""
                xri = xri_pool.tile([74, FREE_IN], dt.int8)
                nc.sync.dma_start(xri[0:2, :], ones_d[0:2, 0:FREE_IN])
                # dst partition 2 + 18g + p  <-  x[4t+g, 496+p, :]
                nc.gpsimd.dma_start(
                    xri[2:74, :],
                    x_d[4 * t : 4 * t + 4, R0_REM : R0_REM + 18, :],
                )
                xr = xrb_pool.tile([74, FREE_IN], dt.bfloat16)
                nc.vector.tensor_copy(xr[:], xri[:])
                rem_bf[t] = xr

            def emit_remainder_compute(t, widx):
                xr = rem_bf[t]
                ps = psum_pool.tile([128, WD_OUT], dt.float32)
                for n in range(3):
                    base = F_PER_CORE * 3 * LHST_MAIN + (t * 3 + n) * LHST_REM
                    for c in range(4):
                        nc.tensor.matmul(
                            ps[0:LHST_REM, c * N_CHUNK : (c + 1) * N_CHUNK],
                            lhsT_t[0:74, base : base + LHST_REM],
                            xr[:, 4 * n + c * N_CHUNK : 4 * n + c * N_CHUNK + N_CHUNK],
                            start=(n == 0),
                            stop=(n == 2),
                            skip_group_check=True,
                        )
                # one out-DMA: dst y[4t+g, 496+i, :] <- ot[16g+i, :]
                requant_store(
                    ps, LHST_REM, widx,
                    y_d[4 * t : 4 * t + 4, R0_REM:H_OUT, :],
                )

            HALF = XT_COLS // 2  # 4 channels per half-tile
            QUARTER = XT_COLS // 4

            win_bf = {}

            def emit_window_load(wi, parts):
                """Load + cast window wi in `parts` column chunks."""
                r0, m_r = MAIN_WINDOWS[wi]
                xi = xi_pool.tile([128, XT_COLS], dt.int8)
                xt = xb_pool.tile([128, XT_COLS], dt.bfloat16)
                nc.sync.dma_start(xi[0:2, :], ones_d[:])
                step = XT_COLS // parts
                chf = F_PER_CORE // parts  # channels per chunk
                for h in range(parts):
                    nc.gpsimd.dma_start(
                        xi[2 : 2 + m_r + 2, h * step : (h + 1) * step],
                        x_d[chf * h : chf * (h + 1), r0 : r0 + m_r + 2, :].transpose(
                            [1, 0, 2]
                        ),
                    )
                    nc.vector.tensor_copy(
                        xt[:, h * step : (h + 1) * step],
                        xi[:, h * step : (h + 1) * step],
                    )
                win_bf[wi] = xt

            def emit_window_compute(wi, widx, last=False):
                r0, m_r = MAIN_WINDOWS[wi]
                xt = win_bf[wi]
                for f in range(F_PER_CORE):
                    ps = psum_pool.tile([128, WD_OUT], dt.float32)
                    for n in range(3):
                        base = (f * 3 + n) * LHST_MAIN
                        xoff = f * X_BLOCK + 4 * n
                        for c in range(4):
                            nc.tensor.matmul(
                                ps[:, c * N_CHUNK : (c + 1) * N_CHUNK],
                                lhsT_t[:, base : base + LHST_MAIN],
                                xt[:, xoff + c * N_CHUNK : xoff + c * N_CHUNK + N_CHUNK],
                                start=(n == 0),
                                stop=(n == 2),
                                skip_group_check=True,
                            )
                    requant_store(
                        ps, m_r, widx + f, y_d[f, r0 : r0 + m_r, :],
                        dve=(last and f % 2 == 1),
                    )
                return widx + F_PER_CORE

            # software pipeline: loads run 1-2 windows ahead of compute
            widx = 0
            emit_window_load(0, 4)
            emit_window_load(1, 2)
            emit_remainder_load(0)
            emit_remainder_load(1)
            widx = emit_window_compute(0, widx)
            emit_window_load(2, 2)
            widx = emit_window_compute(1, widx)
            emit_remainder_compute(0, widx)
            widx += 1
            emit_window_load(3, 2)
            widx = emit_window_compute(2, widx)
            emit_remainder_compute(1, widx)
            widx += 1
            widx = emit_window_compute(3, widx, last=True)

    nc.compile()
    return nc


def make_in_maps(x: np.ndarray, w: np.ndarray, b: np.ndarray):
    ones2 = np.ones((2, XT_COLS), dtype=np.int8)
    in_maps = []
    for core in range(N_CORES):
        lo = core * F_PER_CORE
        hi = lo + F_PER_CORE
        x_shard = np.ascontiguousarray(x[lo:hi]).reshape(F_PER_CORE, H_IN, FREE_IN)
        lhsT = _build_lhsT(w[lo:hi], b[lo:hi])
        in_maps.append({"x": x_shard, "lhsT": lhsT, "ones2": ones2})
    return in_maps


def kernel(x: np.ndarray, w: np.ndarray, b: np.ndarray) -> np.ndarray:
    """x: int8 [64, 514, 514, 4]; w: int8 [64, 3, 3, 1]; b: int32 [64].

    Returns int8 [64, 512, 512, 4].
    """
    from concourse.bass_utils import run_bass_kernel_spmd

    if "nc" not in _PROGRAM_CACHE:
        _PROGRAM_CACHE["nc"] = _build_program()
    nc = _PROGRAM_CACHE["nc"]

    F = x.shape[0]
    assert F == N_CORES * F_PER_CORE

    res = run_bass_kernel_spmd(
        nc, make_in_maps(x, w, b), core_ids=list(range(N_CORES))
    )

    out = np.empty((F, H_OUT, 512, D), dtype=np.int8)
    for core in range(N_CORES):
        lo = core * F_PER_CORE
        y = res.results[core]["y"]  # [8, 512, 2048] int8
        out[lo : lo + F_PER_CORE] = y.reshape(F_PER_CORE, H_OUT, 512, D)
    return out


# revision 21
# speedup vs baseline: 1.4331x; 1.0949x over previous
"""Trainium2 Bass kernel for nn_Conv2D_80796924772741.

Depthwise (grouped, F=64) 3x3 valid conv over [F, 514, 514, 4] int8 with
per-channel int8 weights + int32 bias, followed by exact fixed-point requant
  res = (acc * 19920 + 2^21) >> 22 ;  out = clip(res - 5, -128, 127) int8
(reduced_mantissa 19920 = 1245 * 16 -> res = (acc*1245 + 2^17) >> 18).

Sharding: F=64 split across 8 NeuronCores (8 channels each), embarrassingly
parallel.

Per-core structure (v3):
 - x is loaded window-at-a-time for ALL 8 channels in ONE plain int8 SWDGE
   DMA (the int8->bf16 cast-during-DMA path measured only ~14 GB/s/engine,
   so the cast is done on-chip instead): int8 tile [128, 8*2056], partitions
   0..1 all-ones (int8 ones input; they drive the bias rows), partitions
   2..127 the 126 window rows, channel f at free block f*2056.
 - One whole-tile cast op int8 -> bf16 per window, alternating ACT / DVE.
 - PE: per (channel, window) a [128-row, 2048] PSUM mega-tile (4 banks)
   accumulated by 12 matmuls: Toeplitz-band stationary lhsT (3 H-taps as
   band diagonals, K = 128 incl. 2 bias rows), 3 W-taps as rhs free offset
   +4n, 4 x 512-column chunks.  Bands padded to 128 columns (FWL).
 - Last 16 output rows: 4 channels packed per matmul via block-diagonal
   bands ([74, 64] with shared ones rows), 2 groups.
 - Requant in ONE op per (channel, window) reading the 4-bank PSUM AP:
     out = sat_i8(rne(A * C + B0)),  C = f32(1245/2^18), B0 = f32(-5+4*2^-21)
   alternating ACT / DVE.  Verified bit-exact offline over every
   A in [-147304, 147304] and on-hardware on both engines.
 - Output DMAs alternate SWDGE (gpsimd) / HWDGE (sync) so the y-writes
   spread across all 16 SDMA engines (HWDGE alone was observed to use 4).
"""

import numpy as np
import ml_dtypes

F_PER_CORE = 8
H_IN = 514
W_IN = 514
D = 4
H_OUT = 512
WD_OUT = 2048  # 512 * 4
FREE_IN = W_IN * D  # 2056
N_CHUNK = 512
N_CORES = 8
M_WIN = 124
MAIN_WINDOWS = [(0, 124), (124, 124), (248, 124), (372, 124)]
R0_REM = 496  # remainder: output rows 496..511, 16 per channel
X_BLOCK = FREE_IN
XT_COLS = F_PER_CORE * X_BLOCK  # 16448

LHST_MAIN = 128  # padded band width (FWL) per (f, n) block
LHST_REM = 64  # block-diag remainder width per (group, n) block
LHST_COLS = F_PER_CORE * 3 * LHST_MAIN + 2 * 3 * LHST_REM  # 3456

CPF = np.float32(1245.0 / 2.0**18)
B0 = float(np.float32(-5.0 + 4 * 2.0**-21))


def _build_lhsT(w_core: np.ndarray, b_core: np.ndarray) -> np.ndarray:
    """[128, LHST_COLS] bf16 stationary weights (see module docstring)."""
    out = np.zeros((128, LHST_COLS), dtype=np.float32)
    idx = np.arange(M_WIN)
    for f in range(F_PER_CORE):
        b_f = int(b_core[f])
        bh, bl = b_f >> 3, b_f - 8 * (b_f >> 3)
        for n in range(3):
            base = (f * 3 + n) * LHST_MAIN
            if n == 0:
                out[0, base : base + M_WIN] = float(8 * bh)
                out[1, base : base + M_WIN] = float(bl)
            for m in range(3):
                out[2 + idx + m, base + idx] = float(int(w_core[f, m, n, 0]))
    i16 = np.arange(16)
    for t in range(2):
        for n in range(3):
            base = F_PER_CORE * 3 * LHST_MAIN + (t * 3 + n) * LHST_REM
            for g in range(4):
                f = 4 * t + g
                b_f = int(b_core[f])
                bh, bl = b_f >> 3, b_f - 8 * (b_f >> 3)
                col = base + 16 * g
                if n == 0:
                    out[0, col : col + 16] = float(8 * bh)
                    out[1, col : col + 16] = float(bl)
                for m in range(3):
                    out[2 + 18 * g + i16 + m, col + i16] = float(int(w_core[f, m, n, 0]))
    return out.astype(ml_dtypes.bfloat16)


_PROGRAM_CACHE = {}


def _build_program():
    import concourse.bass as bass
    import concourse.tile as tile
    from concourse import bacc, mybir

    nc = bacc.Bacc(
        "TRN2", target_bir_lowering=False, debug=False, num_devices=N_CORES
    )
    dt = mybir.dt
    Alu = mybir.AluOpType
    Act = mybir.ActivationFunctionType

    x_d = nc.dram_tensor(
        "x", [F_PER_CORE, H_IN, FREE_IN], dt.int8, kind="ExternalInput"
    ).ap()
    lhsT_d = nc.dram_tensor(
        "lhsT", [128, LHST_COLS], dt.bfloat16, kind="ExternalInput"
    ).ap()
    ones_d = nc.dram_tensor(
        "ones2", [2, XT_COLS], dt.int8, kind="ExternalInput"
    ).ap()
    y_d = nc.dram_tensor(
        "y", [F_PER_CORE, H_OUT, WD_OUT], dt.int8, kind="ExternalOutput"
    ).ap()

    with tile.TileContext(nc) as tc:
        with (
            tc.tile_pool(name="const", bufs=1) as const_pool,
            tc.tile_pool(name="xi8", bufs=3) as xi_pool,
            tc.tile_pool(name="xbf", bufs=3) as xb_pool,
            tc.tile_pool(name="xremi", bufs=2) as xri_pool,
            tc.tile_pool(name="xremb", bufs=2) as xrb_pool,
            tc.tile_pool(name="psum", bufs=2, space="PSUM") as psum_pool,
            tc.tile_pool(name="otile", bufs=6) as o_pool,
        ):
            lhsT_t = const_pool.tile([128, LHST_COLS], dt.bfloat16)
            nc.sync.dma_start(lhsT_t[:], lhsT_d[:])

            def requant_store(ps, m_hi, widx, dst_ap, dve=False):
                """One-op requant of ps[0:m_hi, :] -> int8 tile -> DMA out.

                Requants run on ACT (casts own the DVE queue; mixing them
                head-of-line blocks the next window's cast behind a stalled
                requant in the strict-FIFO engine queue).  The final window
                splits ACT/DVE (dve=True) to halve the drain tail.
                """
                ot = o_pool.tile([128, WD_OUT], dt.int8)
                if dve:
                    nc.vector.tensor_scalar(
                        ot[0:m_hi, :], ps[0:m_hi, :], float(CPF), B0,
                        Alu.mult, Alu.add,
                    )
                else:
                    nc.scalar.activation(
                        ot[0:m_hi, :], ps[0:m_hi, :], Act.Copy,
                        bias=B0, scale=float(CPF),
                    )
                # outs on SWDGE: it spreads over all 16 SDMA engines, and
                # keeps the HWDGE ring free for the x loads
                nc.gpsimd.dma_start(dst_ap, ot[0:m_hi, :])

            rem_bf = {}

            def emit_remainder_load(t):
                """Load + cast for remainder rows (496..511, channel group t)."""
                xri = xri_pool.tile([74, FREE_IN], dt.int8)
                nc.sync.dma_start(xri[0:2, :], ones_d[0:2, 0:FREE_IN])
                # dst partition 2 + 18g + p  <-  x[4t+g, 496+p, :]
                nc.sync.dma_start(
                    xri[2:74, :],
                    x_d[4 * t : 4 * t + 4, R0_REM : R0_REM + 18, :],
                )
                xr = xrb_pool.tile([74, FREE_IN], dt.bfloat16)
                nc.vector.tensor_copy(xr[:], xri[:])
                rem_bf[t] = xr

            def emit_remainder_compute(t, widx):
                xr = rem_bf[t]
                ps = psum_pool.tile([128, WD_OUT], dt.float32)
                for n in range(3):
                    base = F_PER_CORE * 3 * LHST_MAIN + (t * 3 + n) * LHST_REM
                    for c in range(4):
                        nc.tensor.matmul(
                            ps[0:LHST_REM, c * N_CHUNK : (c + 1) * N_CHUNK],
                            lhsT_t[0:74, base : base + LHST_REM],
                            xr[:, 4 * n + c * N_CHUNK : 4 * n + c * N_CHUNK + N_CHUNK],
                            start=(n == 0),
                            stop=(n == 2),
                            skip_group_check=True,
                        )
                # one out-DMA: dst y[4t+g, 496+i, :] <- ot[16g+i, :]
                requant_store(
                    ps, LHST_REM, widx,
                    y_d[4 * t : 4 * t + 4, R0_REM:H_OUT, :],
                )

            HALF = XT_COLS // 2  # 4 channels per half-tile
            QUARTER = XT_COLS // 4

            win_bf = {}

            def emit_window_load(wi, parts):
                """Load + cast window wi in `parts` column chunks."""
                r0, m_r = MAIN_WINDOWS[wi]
                xi = xi_pool.tile([128, XT_COLS], dt.int8)
                xt = xb_pool.tile([128, XT_COLS], dt.bfloat16)
                nc.sync.dma_start(xi[0:2, :], ones_d[:])
                step = XT_COLS // parts
                chf = F_PER_CORE // parts  # channels per chunk
                for h in range(parts):
                    nc.sync.dma_start(
                        xi[2 : 2 + m_r + 2, h * step : (h + 1) * step],
                        x_d[chf * h : chf * (h + 1), r0 : r0 + m_r + 2, :].transpose(
                            [1, 0, 2]
                        ),
                    )
                    nc.vector.tensor_copy(
                        xt[:, h * step : (h + 1) * step],
                        xi[:, h * step : (h + 1) * step],
                    )
                win_bf[wi] = xt

            def emit_window_compute(wi, widx, last=False):
                r0, m_r = MAIN_WINDOWS[wi]
                xt = win_bf[wi]
                for f in range(F_PER_CORE):
                    ps = psum_pool.tile([128, WD_OUT], dt.float32)
                    if wi == 0 and f == 0:
                        # PE warmup: ~35 small matmuls on the already-loaded
                        # lhsT keep the PE busy through one HAM SHORT window
                        # during the first x loads, so real matmuls start at
                        # 2.4 GHz.  They write this tile's first bank, which
                        # the real n=0 matmul (start=True) clears anyway.
                        for _ in range(35):
                            nc.tensor.matmul(
                                ps[:, 0:128],
                                lhsT_t[:, 0:128],
                                lhsT_t[:, 0:128],
                                start=True,
                                stop=True,
                                skip_group_check=True,
                            )
                    if not last:
                        for n in range(3):
                            base = (f * 3 + n) * LHST_MAIN
                            xoff = f * X_BLOCK + 4 * n
                            for c in range(4):
                                nc.tensor.matmul(
                                    ps[:, c * N_CHUNK : (c + 1) * N_CHUNK],
                                    lhsT_t[:, base : base + LHST_MAIN],
                                    xt[:, xoff + c * N_CHUNK : xoff + c * N_CHUNK + N_CHUNK],
                                    start=(n == 0),
                                    stop=(n == 2),
                                    skip_group_check=True,
                                )
                        requant_store(
                            ps, m_r, widx + f, y_d[f, r0 : r0 + m_r, :]
                        )
                    else:
                        # final window: chunk-outer so each 512-col chunk
                        # requants + stores as soon as its 3 taps land,
                        # alternating ACT/DVE — shrinks the drain tail
                        ot = o_pool.tile([128, WD_OUT], dt.int8)
                        for c in range(4):
                            for n in range(3):
                                base = (f * 3 + n) * LHST_MAIN
                                xoff = f * X_BLOCK + 4 * n
                                nc.tensor.matmul(
                                    ps[:, c * N_CHUNK : (c + 1) * N_CHUNK],
                                    lhsT_t[:, base : base + LHST_MAIN],
                                    xt[:, xoff + c * N_CHUNK : xoff + c * N_CHUNK + N_CHUNK],
                                    start=(n == 0),
                                    stop=(n == 2),
                                    skip_group_check=True,
                                )
                            cs = slice(c * N_CHUNK, (c + 1) * N_CHUNK)
                            if (f + c) % 2 == 0:
                                nc.scalar.activation(
                                    ot[0:m_r, cs], ps[0:m_r, cs], Act.Copy,
                                    bias=B0, scale=float(CPF),
                                )
                            else:
                                nc.vector.tensor_scalar(
                                    ot[0:m_r, cs], ps[0:m_r, cs], float(CPF), B0,
                                    Alu.mult, Alu.add,
                                )
                            if c % 2 == 0:
                                nc.gpsimd.dma_start(
                                    y_d[f, r0 : r0 + m_r, cs], ot[0:m_r, cs]
                                )
                            else:
                                nc.sync.dma_start(
                                    y_d[f, r0 : r0 + m_r, cs], ot[0:m_r, cs]
                                )
                return widx + F_PER_CORE

            # software pipeline: all loads early, casts 1-2 windows ahead
            widx = 0
            emit_window_load(0, 4)
            emit_window_load(1, 2)
            emit_window_load(2, 2)
            emit_remainder_load(0)
            emit_remainder_load(1)
            widx = emit_window_compute(0, widx)
            emit_window_load(3, 2)
            widx = emit_window_compute(1, widx)
            emit_remainder_compute(0, widx)
            widx += 1
            widx = emit_window_compute(2, widx)
            emit_remainder_compute(1, widx)
            widx += 1
            widx = emit_window_compute(3, widx, last=True)

    nc.compile()
    return nc


def make_in_maps(x: np.ndarray, w: np.ndarray, b: np.ndarray):
    ones2 = np.ones((2, XT_COLS), dtype=np.int8)
    in_maps = []
    for core in range(N_CORES):
        lo = core * F_PER_CORE
        hi = lo + F_PER_CORE
        x_shard = np.ascontiguousarray(x[lo:hi]).reshape(F_PER_CORE, H_IN, FREE_IN)
        lhsT = _build_lhsT(w[lo:hi], b[lo:hi])
        in_maps.append({"x": x_shard, "lhsT": lhsT, "ones2": ones2})
    return in_maps


def kernel(x: np.ndarray, w: np.ndarray, b: np.ndarray) -> np.ndarray:
    """x: int8 [64, 514, 514, 4]; w: int8 [64, 3, 3, 1]; b: int32 [64].

    Returns int8 [64, 512, 512, 4].
    """
    from concourse.bass_utils import run_bass_kernel_spmd

    if "nc" not in _PROGRAM_CACHE:
        _PROGRAM_CACHE["nc"] = _build_program()
    nc = _PROGRAM_CACHE["nc"]

    F = x.shape[0]
    assert F == N_CORES * F_PER_CORE

    res = run_bass_kernel_spmd(
        nc, make_in_maps(x, w, b), core_ids=list(range(N_CORES))
    )

    out = np.empty((F, H_OUT, 512, D), dtype=np.int8)
    for core in range(N_CORES):
        lo = core * F_PER_CORE
        y = res.results[core]["y"]  # [8, 512, 2048] int8
        out[lo : lo + F_PER_CORE] = y.reshape(F_PER_CORE, H_OUT, 512, D)
    return out


# revision 23
# speedup vs baseline: 1.4810x; 1.0334x over previous
"""Trainium2 Bass kernel for nn_Conv2D_80796924772741.

Depthwise (grouped, F=64) 3x3 valid conv over [F, 514, 514, 4] int8 with
per-channel int8 weights + int32 bias, followed by exact fixed-point requant
  res = (acc * 19920 + 2^21) >> 22 ;  out = clip(res - 5, -128, 127) int8
(reduced_mantissa 19920 = 1245 * 16 -> res = (acc*1245 + 2^17) >> 18).

Sharding: F=64 split across 8 NeuronCores (8 channels each), embarrassingly
parallel.

Per-core structure (v3):
 - x is loaded window-at-a-time for ALL 8 channels in ONE plain int8 SWDGE
   DMA (the int8->bf16 cast-during-DMA path measured only ~14 GB/s/engine,
   so the cast is done on-chip instead): int8 tile [128, 8*2056], partitions
   0..1 all-ones (int8 ones input; they drive the bias rows), partitions
   2..127 the 126 window rows, channel f at free block f*2056.
 - One whole-tile cast op int8 -> bf16 per window, alternating ACT / DVE.
 - PE: per (channel, window) a [128-row, 2048] PSUM mega-tile (4 banks)
   accumulated by 12 matmuls: Toeplitz-band stationary lhsT (3 H-taps as
   band diagonals, K = 128 incl. 2 bias rows), 3 W-taps as rhs free offset
   +4n, 4 x 512-column chunks.  Bands padded to 128 columns (FWL).
 - Last 16 output rows: 4 channels packed per matmul via block-diagonal
   bands ([74, 64] with shared ones rows), 2 groups.
 - Requant in ONE op per (channel, window) reading the 4-bank PSUM AP:
     out = sat_i8(rne(A * C + B0)),  C = f32(1245/2^18), B0 = f32(-5+4*2^-21)
   alternating ACT / DVE.  Verified bit-exact offline over every
   A in [-147304, 147304] and on-hardware on both engines.
 - Output DMAs alternate SWDGE (gpsimd) / HWDGE (sync) so the y-writes
   spread across all 16 SDMA engines (HWDGE alone was observed to use 4).
"""

import numpy as np
import ml_dtypes

F_PER_CORE = 8
H_IN = 514
W_IN = 514
D = 4
H_OUT = 512
WD_OUT = 2048  # 512 * 4
FREE_IN = W_IN * D  # 2056
N_CHUNK = 512
N_CORES = 8
M_WIN = 124
MAIN_WINDOWS = [(0, 124), (124, 124), (248, 124), (372, 124)]
R0_REM = 496  # remainder: output rows 496..511, 16 per channel
X_BLOCK = FREE_IN
XT_COLS = F_PER_CORE * X_BLOCK  # 16448

LHST_MAIN = 128  # padded band width (FWL) per (f, n) block
LHST_REM = 64  # block-diag remainder width per (group, n) block
LHST_COLS = F_PER_CORE * 3 * LHST_MAIN + 2 * 3 * LHST_REM  # 3456

CPF = np.float32(1245.0 / 2.0**18)
B0 = float(np.float32(-5.0 + 4 * 2.0**-21))


def _build_lhsT(w_core: np.ndarray, b_core: np.ndarray) -> np.ndarray:
    """[128, LHST_COLS] bf16 stationary weights (see module docstring)."""
    out = np.zeros((128, LHST_COLS), dtype=np.float32)
    idx = np.arange(M_WIN)
    for f in range(F_PER_CORE):
        b_f = int(b_core[f])
        bh, bl = b_f >> 3, b_f - 8 * (b_f >> 3)
        for n in range(3):
            base = (f * 3 + n) * LHST_MAIN
            if n == 0:
                out[0, base : base + M_WIN] = float(8 * bh)
                out[1, base : base + M_WIN] = float(bl)
            for m in range(3):
                out[2 + idx + m, base + idx] = float(int(w_core[f, m, n, 0]))
    i16 = np.arange(16)
    for t in range(2):
        for n in range(3):
            base = F_PER_CORE * 3 * LHST_MAIN + (t * 3 + n) * LHST_REM
            for g in range(4):
                f = 4 * t + g
                b_f = int(b_core[f])
                bh, bl = b_f >> 3, b_f - 8 * (b_f >> 3)
                col = base + 16 * g
                if n == 0:
                    out[0, col : col + 16] = float(8 * bh)
                    out[1, col : col + 16] = float(bl)
                for m in range(3):
                    out[2 + 18 * g + i16 + m, col + i16] = float(int(w_core[f, m, n, 0]))
    return out.astype(ml_dtypes.bfloat16)


_PROGRAM_CACHE = {}


def _build_program():
    import concourse.bass as bass
    import concourse.tile as tile
    from concourse import bacc, mybir

    nc = bacc.Bacc(
        "TRN2", target_bir_lowering=False, debug=False, num_devices=N_CORES
    )
    dt = mybir.dt
    Alu = mybir.AluOpType
    Act = mybir.ActivationFunctionType

    x_d = nc.dram_tensor(
        "x", [F_PER_CORE, H_IN, FREE_IN], dt.int8, kind="ExternalInput"
    ).ap()
    lhsT_d = nc.dram_tensor(
        "lhsT", [128, LHST_COLS], dt.bfloat16, kind="ExternalInput"
    ).ap()
    ones_d = nc.dram_tensor(
        "ones2", [2, XT_COLS], dt.int8, kind="ExternalInput"
    ).ap()
    y_d = nc.dram_tensor(
        "y", [F_PER_CORE, H_OUT, WD_OUT], dt.int8, kind="ExternalOutput"
    ).ap()

    with tile.TileContext(nc) as tc:
        with (
            tc.tile_pool(name="const", bufs=1) as const_pool,
            tc.tile_pool(name="xi8", bufs=3) as xi_pool,
            tc.tile_pool(name="xbf", bufs=3) as xb_pool,
            tc.tile_pool(name="xremi", bufs=2) as xri_pool,
            tc.tile_pool(name="xremb", bufs=2) as xrb_pool,
            tc.tile_pool(name="psum", bufs=2, space="PSUM") as psum_pool,
            tc.tile_pool(name="otile", bufs=6) as o_pool,
        ):
            lhsT_t = const_pool.tile([128, LHST_COLS], dt.bfloat16)
            nc.sync.dma_start(lhsT_t[:], lhsT_d[:])

            def requant_store(ps, m_hi, widx, dst_ap, dve=False):
                """One-op requant of ps[0:m_hi, :] -> int8 tile -> DMA out.

                Requants run on ACT (casts own the DVE queue; mixing them
                head-of-line blocks the next window's cast behind a stalled
                requant in the strict-FIFO engine queue).  The final window
                splits ACT/DVE (dve=True) to halve the drain tail.
                """
                ot = o_pool.tile([128, WD_OUT], dt.int8)
                if dve:
                    nc.vector.tensor_scalar(
                        ot[0:m_hi, :], ps[0:m_hi, :], float(CPF), B0,
                        Alu.mult, Alu.add,
                    )
                else:
                    nc.scalar.activation(
                        ot[0:m_hi, :], ps[0:m_hi, :], Act.Copy,
                        bias=B0, scale=float(CPF),
                    )
                # outs on SWDGE: it spreads over all 16 SDMA engines, and
                # keeps the HWDGE ring free for the x loads
                nc.gpsimd.dma_start(dst_ap, ot[0:m_hi, :])

            rem_bf = {}

            def emit_remainder_load(t):
                """Load + cast for remainder rows (496..511, channel group t)."""
                xri = xri_pool.tile([74, FREE_IN], dt.int8)
                nc.sync.dma_start(xri[0:2, :], ones_d[0:2, 0:FREE_IN])
                # dst partition 2 + 18g + p  <-  x[4t+g, 496+p, :]
                nc.sync.dma_start(
                    xri[2:74, :],
                    x_d[4 * t : 4 * t + 4, R0_REM : R0_REM + 18, :],
                )
                xr = xrb_pool.tile([74, FREE_IN], dt.bfloat16)
                nc.vector.tensor_copy(xr[:], xri[:])
                rem_bf[t] = xr

            def emit_remainder_compute(t, widx):
                xr = rem_bf[t]
                ps = psum_pool.tile([128, WD_OUT], dt.float32)
                for n in range(3):
                    base = F_PER_CORE * 3 * LHST_MAIN + (t * 3 + n) * LHST_REM
                    for c in range(4):
                        nc.tensor.matmul(
                            ps[0:LHST_REM, c * N_CHUNK : (c + 1) * N_CHUNK],
                            lhsT_t[0:74, base : base + LHST_REM],
                            xr[:, 4 * n + c * N_CHUNK : 4 * n + c * N_CHUNK + N_CHUNK],
                            start=(n == 0),
                            stop=(n == 2),
                            skip_group_check=True,
                        )
                # one out-DMA: dst y[4t+g, 496+i, :] <- ot[16g+i, :]
                # rem0 on ACT+gpsimd, rem1 on DVE+sync -> the two tail
                # groups requant and store fully in parallel
                ot = o_pool.tile([128, WD_OUT], dt.int8)
                dst_ap = y_d[4 * t : 4 * t + 4, R0_REM:H_OUT, :]
                if t == 0:
                    nc.scalar.activation(
                        ot[0:LHST_REM, :], ps[0:LHST_REM, :], Act.Copy,
                        bias=B0, scale=float(CPF),
                    )
                    nc.gpsimd.dma_start(dst_ap, ot[0:LHST_REM, :])
                else:
                    nc.vector.tensor_scalar(
                        ot[0:LHST_REM, :], ps[0:LHST_REM, :], float(CPF), B0,
                        Alu.mult, Alu.add,
                    )
                    nc.sync.dma_start(dst_ap, ot[0:LHST_REM, :])

            HALF = XT_COLS // 2  # 4 channels per half-tile
            QUARTER = XT_COLS // 4

            win_bf = {}

            def emit_window_load(wi, parts):
                """Load + cast window wi in `parts` column chunks."""
                r0, m_r = MAIN_WINDOWS[wi]
                xi = xi_pool.tile([128, XT_COLS], dt.int8)
                xt = xb_pool.tile([128, XT_COLS], dt.bfloat16)
                nc.sync.dma_start(xi[0:2, :], ones_d[:])
                step = XT_COLS // parts
                chf = F_PER_CORE // parts  # channels per chunk
                for h in range(parts):
                    nc.sync.dma_start(
                        xi[2 : 2 + m_r + 2, h * step : (h + 1) * step],
                        x_d[chf * h : chf * (h + 1), r0 : r0 + m_r + 2, :].transpose(
                            [1, 0, 2]
                        ),
                    )
                    nc.vector.tensor_copy(
                        xt[:, h * step : (h + 1) * step],
                        xi[:, h * step : (h + 1) * step],
                    )
                win_bf[wi] = xt

            def emit_window_compute(wi, widx, last=False):
                r0, m_r = MAIN_WINDOWS[wi]
                xt = win_bf[wi]
                for f in range(F_PER_CORE):
                    ps = psum_pool.tile([128, WD_OUT], dt.float32)
                    if wi == 0 and f == 0:
                        # PE warmup: ~35 small matmuls on the already-loaded
                        # lhsT keep the PE busy through one HAM SHORT window
                        # during the first x loads, so real matmuls start at
                        # 2.4 GHz.  They write this tile's first bank, which
                        # the real n=0 matmul (start=True) clears anyway.
                        for _ in range(35):
                            nc.tensor.matmul(
                                ps[:, 0:128],
                                lhsT_t[:, 0:128],
                                lhsT_t[:, 0:128],
                                start=True,
                                stop=True,
                                skip_group_check=True,
                            )
                    if not last:
                        for n in range(3):
                            base = (f * 3 + n) * LHST_MAIN
                            xoff = f * X_BLOCK + 4 * n
                            for c in range(4):
                                nc.tensor.matmul(
                                    ps[:, c * N_CHUNK : (c + 1) * N_CHUNK],
                                    lhsT_t[:, base : base + LHST_MAIN],
                                    xt[:, xoff + c * N_CHUNK : xoff + c * N_CHUNK + N_CHUNK],
                                    start=(n == 0),
                                    stop=(n == 2),
                                    skip_group_check=True,
                                )
                        requant_store(
                            ps, m_r, widx + f, y_d[f, r0 : r0 + m_r, :]
                        )
                    else:
                        # final window: chunk-outer so each 512-col chunk
                        # requants + stores as soon as its 3 taps land,
                        # alternating ACT/DVE — shrinks the drain tail
                        ot = o_pool.tile([128, WD_OUT], dt.int8)
                        for c in range(4):
                            for n in range(3):
                                base = (f * 3 + n) * LHST_MAIN
                                xoff = f * X_BLOCK + 4 * n
                                nc.tensor.matmul(
                                    ps[:, c * N_CHUNK : (c + 1) * N_CHUNK],
                                    lhsT_t[:, base : base + LHST_MAIN],
                                    xt[:, xoff + c * N_CHUNK : xoff + c * N_CHUNK + N_CHUNK],
                                    start=(n == 0),
                                    stop=(n == 2),
                                    skip_group_check=True,
                                )
                            cs = slice(c * N_CHUNK, (c + 1) * N_CHUNK)
                            if (f + c) % 2 == 0:
                                nc.scalar.activation(
                                    ot[0:m_r, cs], ps[0:m_r, cs], Act.Copy,
                                    bias=B0, scale=float(CPF),
                                )
                            else:
                                nc.vector.tensor_scalar(
                                    ot[0:m_r, cs], ps[0:m_r, cs], float(CPF), B0,
                                    Alu.mult, Alu.add,
                                )
                            if c % 2 == 0:
                                nc.gpsimd.dma_start(
                                    y_d[f, r0 : r0 + m_r, cs], ot[0:m_r, cs]
                                )
                            else:
                                nc.sync.dma_start(
                                    y_d[f, r0 : r0 + m_r, cs], ot[0:m_r, cs]
                                )
                return widx + F_PER_CORE

            # software pipeline: all loads early, casts 1-2 windows ahead;
            # remainders (tiny outputs) computed last so the big final-window
            # output drains while their matmuls run
            widx = 0
            emit_window_load(0, 8)
            emit_window_load(1, 2)
            emit_window_load(2, 2)
            emit_remainder_load(0)
            emit_remainder_load(1)
            widx = emit_window_compute(0, widx)
            emit_window_load(3, 2)
            widx = emit_window_compute(1, widx)
            widx = emit_window_compute(2, widx)
            widx = emit_window_compute(3, widx, last=True)
            emit_remainder_compute(0, widx)
            widx += 1
            emit_remainder_compute(1, widx)
            widx += 1

    nc.compile()
    return nc


def make_in_maps(x: np.ndarray, w: np.ndarray, b: np.ndarray):
    ones2 = np.ones((2, XT_COLS), dtype=np.int8)
    in_maps = []
    for core in range(N_CORES):
        lo = core * F_PER_CORE
        hi = lo + F_PER_CORE
        x_shard = np.ascontiguousarray(x[lo:hi]).reshape(F_PER_CORE, H_IN, FREE_IN)
        lhsT = _build_lhsT(w[lo:hi], b[lo:hi])
        in_maps.append({"x": x_shard, "lhsT": lhsT, "ones2": ones2})
    return in_maps


def kernel(x: np.ndarray, w: np.ndarray, b: np.ndarray) -> np.ndarray:
    """x: int8 [64, 514, 514, 4]; w: int8 [64, 3, 3, 1]; b: int32 [64].

    Returns int8 [64, 512, 512, 4].
    """
    from concourse.bass_utils import run_bass_kernel_spmd

    if "nc" not in _PROGRAM_CACHE:
        _PROGRAM_CACHE["nc"] = _build_program()
    nc = _PROGRAM_CACHE["nc"]

    F = x.shape[0]
    assert F == N_CORES * F_PER_CORE

    res = run_bass_kernel_spmd(
        nc, make_in_maps(x, w, b), core_ids=list(range(N_CORES))
    )

    out = np.empty((F, H_OUT, 512, D), dtype=np.int8)
    for core in range(N_CORES):
        lo = core * F_PER_CORE
        y = res.results[core]["y"]  # [8, 512, 2048] int8
        out[lo : lo + F_PER_CORE] = y.reshape(F_PER_CORE, H_OUT, 512, D)
    return out
